# revision 9
# baseline (speedup 1.0000x reference)
"""Trainium2 Bass kernel for the dense-MoE nn module (nn_MoE_13537736917214).

Strategy
--------
Data-parallel over 8 NeuronCores: each core takes an 8192-token slice of x
and the full (replicated) expert + gating weights, computes its slice of y
fully fused in SBUF/PSUM (the jax reference is HBM-bound because it
materializes [E, N, H] activations; we never spill them), and the host
concatenates the 8 output shards.

Per-core structure: 4 blocks x 2048 tokens, matmul chunks of 512 tokens
(one PSUM bank). Activations live as [feature, token] tiles (feature on
partitions). All matmuls run in float32r (~1e-4 relative error, full
1 cycle/row PE rate at moving size >= 256). Bias handling:
  - expert/gating first layer: ones-row augmented stationary (K=I+1=3)
  - expert middle layers: ScalarE activation's free per-partition bias-add
  - gating output bias gbo: folded into the sin range-reduction DVE op
  - expert output bias ebo: PSUM prefill via a K=1 matmul against the
    ones row, accumulated with the per-expert output dots
Gates sin(5*pi*z): HW Sin is only accurate on [-pi, pi], so range-reduce:
w = 2.5*z', frac = w - int32_round(w), gates = Sin(2*pi*frac).
Per-expert output dot: masked stationaries [128, 16] (column e holds
ewo[e], others zero) accumulate all 16 experts into one [16, 2048] PSUM
tile; combine = DVE eo*gates + ones-vector reduce matmul.
"""

import sys

if "/opt/trn_rl_repo" not in sys.path:
    sys.path.insert(0, "/opt/trn_rl_repo")

import numpy as np

N, E, I, H, O, GH = 65536, 16, 2, 128, 1, 64
NCORES = 8
NTOK = N // NCORES          # tokens per core
TB = 2048                   # token block
CH = 512                    # matmul chunk (one PSUM bank of fp32)
NBLK = NTOK // TB
NCH = TB // CH
PI = float(np.pi)

_CACHE = {}


def _split_excess_waits(nc, mybir, max_waits=1):
    """Walrus CoreV3 rejects TPB instructions with >1 sync-wait command.
    Hoist excess waits onto NoOps inserted before the offender on the same
    engine (engine streams are in-order, so earlier waits are safe)."""
    for func in nc.m.functions:
        for blk in func.blocks:
            insts = blk.instructions
            idx = 0
            while idx < len(insts):
                inst = insts[idx]
                si = inst.sync_info
                if si is not None and len(si.on_wait) > max_waits:
                    waits = list(si.on_wait)
                    si.on_wait = waits[:max_waits]
                    for j in range(max_waits, len(waits), max_waits):
                        nop = mybir.InstNoOp(
                            name=nc.get_next_instruction_name(), ins=[], outs=[]
                        )
                        nop.engine = inst.engine
                        nop.sync_info = mybir.SyncInfo(
                            on_wait=waits[j : j + max_waits], on_update=[]
                        )
                        nc.register_instruction(nop)
                        blk.instructions.insert(idx, nop)
                        idx += 1
                idx += 1


def _build_program():
    import concourse.bass as bass
    import concourse.tile as tile
    from concourse import mybir

    dt = mybir.dt
    A = mybir.AluOpType
    AF = mybir.ActivationFunctionType

    nc = bass.Bass()
    x_d = nc.declare_dram_parameter("x", [NTOK, I], dt.float32, isOutput=False)
    ew1_d = nc.declare_dram_parameter("ew1", [E, I, H], dt.float32, isOutput=False)
    eb1_d = nc.declare_dram_parameter("eb1", [E, H], dt.float32, isOutput=False)
    ewm_d = nc.declare_dram_parameter("ewm", [E, 3, H, H], dt.float32, isOutput=False)
    ebm_d = nc.declare_dram_parameter("ebm", [E, 3, H], dt.float32, isOutput=False)
    ewo_d = nc.declare_dram_parameter("ewo", [E, H, O], dt.float32, isOutput=False)
    ebo_d = nc.declare_dram_parameter("ebo", [E, O], dt.float32, isOutput=False)
    gw1_d = nc.declare_dram_parameter("gw1", [I, GH], dt.float32, isOutput=False)
    gb1_d = nc.declare_dram_parameter("gb1", [GH], dt.float32, isOutput=False)
    gwm_d = nc.declare_dram_parameter("gwm", [2, GH, GH], dt.float32, isOutput=False)
    gbm_d = nc.declare_dram_parameter("gbm", [2, GH], dt.float32, isOutput=False)
    gwo_d = nc.declare_dram_parameter("gwo", [GH, E], dt.float32, isOutput=False)
    gbo_d = nc.declare_dram_parameter("gbo", [E], dt.float32, isOutput=False)
    y_d = nc.declare_dram_parameter("y", [NTOK, O], dt.float32, isOutput=True)

    with tile.TileContext(nc) as tc:
        # ---- persistent weight/constant pools (allocated before staging) ----
        wpool = tc.alloc_tile_pool(name="weights", bufs=1)
        xaug = wpool.tile([3, NTOK], dt.float32r)        # xT rows 0-1, ones row 2
        w1s = wpool.tile([3, E * H], dt.float32r)        # expert L1 aug stationaries
        wms = wpool.tile([H, E * 3 * H], dt.float32r)    # expert middle stationaries
        wos = wpool.tile([H, E * E], dt.bfloat16)        # masked output stationaries
        g1s = wpool.tile([3, GH], dt.float32r)           # gating L1 aug stationary
        gms = wpool.tile([GH, 2 * GH], dt.float32r)      # gating middle stationaries
        gos = wpool.tile([GH, E], dt.float32r)           # gating output stationary
        ebm_sb = wpool.tile([H, E * 3], dt.float32)      # ACT bias vectors
        gbm_sb = wpool.tile([GH, 2], dt.float32)         # gating ACT bias vectors
        gbo25 = wpool.tile([E, 1], dt.float32)           # 2.5 * gbo
        ebo_row = wpool.tile([1, E], dt.float32r)        # ebo as K=1 stationary
        ones16 = wpool.tile([E, 1], dt.float32r)         # reduce stationary
        onesch = wpool.tile([1, CH], dt.float32r)        # ones moving row (prefill)

        # ---- staging: DMA fp32 weights in, round to float32r on DVE ----
        with tc.tile_pool(name="staging", bufs=1) as st:
            xs = st.tile([3, NTOK], dt.float32, tag="big")
            nc.gpsimd.memset(xs[:], 1.0)
            nc.sync.dma_start(xs[0:2, :], x_d[:].rearrange("n i -> i n"))
            nc.vector.tensor_copy(xaug[:], xs[:])
            on = st.tile([1, CH], dt.float32, tag="ones")
            nc.gpsimd.memset(on[:], 1.0)
            nc.vector.tensor_copy(onesch[:], on[:])

            ws = st.tile([H, E * 3 * H], dt.float32, tag="big2")
            nc.sync.dma_start(ws[:].rearrange("h (e i k) -> h e i k", e=E, i=3),
                              ewm_d[:].rearrange("e i h k -> h e i k"))
            nc.vector.tensor_copy(wms[:], ws[:])

            w1 = st.tile([3, E * H], dt.float32, tag="w1")
            nc.sync.dma_start(w1[0:2, :].rearrange("i (e h) -> i e h", e=E),
                              ew1_d[:].rearrange("e i h -> i e h"))
            nc.sync.dma_start(w1[2:3, :], eb1_d[:].rearrange("e h -> (e h)")[None, :])
            nc.vector.tensor_copy(w1s[:], w1[:])

            # masked ewo stationaries: zero, then scatter ewo[e] into col e*16+e
            woz = st.tile([H, E * E], dt.float32, tag="woz")
            nc.gpsimd.memset(woz[:], 0.0)
            wo = st.tile([H, E], dt.float32, tag="wo")
            nc.sync.dma_start(wo[:], ewo_d[:].rearrange("e h o -> h (e o)"))
            for e in range(E):
                nc.vector.tensor_copy(woz[:, e * E + e : e * E + e + 1],
                                      wo[:, e : e + 1])
            nc.vector.tensor_copy(wos[:], woz[:])

            gs = st.tile([GH + 1, 2 * GH + GH + E + 3], dt.float32, tag="gs")
            # gating L1 aug [3, GH]
            nc.sync.dma_start(gs[0:2, 0:GH], gw1_d[:])
            nc.sync.dma_start(gs[2:3, 0:GH], gb1_d[:][None, :])
            nc.vector.tensor_copy(g1s[:], gs[0:3, 0:GH])
            # gating middle [GH, 2*GH]
            nc.sync.dma_start(gs[0:GH, GH : GH + 2 * GH].rearrange("a (i b) -> a i b", i=2),
                              gwm_d[:].rearrange("i a b -> a i b"))
            nc.vector.tensor_copy(gms[:], gs[0:GH, GH : GH + 2 * GH])
            # gating out [GH, E]
            nc.sync.dma_start(gs[0:GH, 3 * GH : 3 * GH + E], gwo_d[:])
            nc.vector.tensor_copy(gos[:], gs[0:GH, 3 * GH : 3 * GH + E])

            # biases
            nc.sync.dma_start(ebm_sb[:].rearrange("h (e i) -> h e i", e=E),
                          ebm_d[:].rearrange("e i h -> h e i"))
            nc.sync.dma_start(gbm_sb[:], gbm_d[:].rearrange("i g -> g i"))
            bo = st.tile([E, 2], dt.float32, tag="bo")
            nc.sync.dma_start(bo[:, 0:1], gbo_d[:][:, None])
            nc.vector.tensor_scalar(gbo25[:], bo[:, 0:1], 2.5, None, A.mult)
            bor = st.tile([1, E], dt.float32, tag="bor")
            nc.sync.dma_start(bor[:], ebo_d[:].rearrange("e o -> o e"))
            nc.vector.tensor_copy(ebo_row[:], bor[:])
            nc.gpsimd.memset(bo[:, 1:2], 1.0)
            nc.vector.tensor_copy(ones16[:], bo[:, 1:2])

        # ---- working pools ----
        hpool = tc.alloc_tile_pool(name="h", bufs=3)
        h4pool = tc.alloc_tile_pool(name="h4", bufs=1)
        gpool = tc.alloc_tile_pool(name="g", bufs=2)
        spool = tc.alloc_tile_pool(name="s", bufs=1)
        pp = tc.alloc_tile_pool(name="pp", bufs=2, space="PSUM")

        h4all = h4pool.tile([H, E * TB], dt.bfloat16)

        for b in range(NBLK):
            t0 = b * TB

            def chunks():
                return [(c, t0 + c * CH) for c in range(NCH)]

            # ---------------- gating ----------------
            z1 = pp.tile([GH, TB], dt.float32, tag="pp")
            for c, t in chunks():
                nc.tensor.matmul(z1[:, c * CH : (c + 1) * CH], g1s[:],
                                 xaug[:, t : t + CH], start=True, stop=True)
            g1 = gpool.tile([GH, TB], dt.float32r, tag="g")
            nc.scalar.activation(g1[:], z1[:], AF.Tanh)

            g_prev = g1
            for i in range(2):
                z = pp.tile([GH, TB], dt.float32, tag="pp")
                for c, t in chunks():
                    nc.tensor.matmul(z[:, c * CH : (c + 1) * CH],
                                     gms[:, i * GH : (i + 1) * GH],
                                     g_prev[:, c * CH : (c + 1) * CH],
                                     start=True, stop=True)
                gn = gpool.tile([GH, TB], dt.float32r, tag="g")
                nc.scalar.activation(gn[:], z[:], AF.Tanh, bias=gbm_sb[:, i : i + 1])
                g_prev = gn

            zg = pp.tile([E, TB], dt.float32, tag="pp")
            for c, t in chunks():
                nc.tensor.matmul(zg[:, c * CH : (c + 1) * CH], gos[:],
                                 g_prev[:, c * CH : (c + 1) * CH],
                                 start=True, stop=True)
            # gates = sin(5*pi*(zg + gbo)) = sin(2*pi*frac(2.5*zg + 2.5*gbo))
            w = spool.tile([E, TB], dt.float32, tag="s0")
            nc.vector.tensor_scalar(w[:], zg[:], 2.5, gbo25[:], A.mult, A.add)
            wi = spool.tile([E, TB], dt.int32, tag="s1")
            nc.vector.tensor_copy(wi[:], w[:])
            fr = spool.tile([E, TB], dt.float32, tag="s2")
            nc.vector.tensor_tensor(fr[:], w[:], wi[:], A.subtract)
            gates = spool.tile([E, TB], dt.float32, tag="s0")
            nc.scalar.activation(gates[:], fr[:], AF.Sin, scale=2.0 * PI)

            # ---------------- experts ----------------
            for e in range(E):
                z = pp.tile([H, TB], dt.float32, tag="pp")
                for c, t in chunks():
                    nc.tensor.matmul(z[:, c * CH : (c + 1) * CH],
                                     w1s[:, e * H : (e + 1) * H],
                                     xaug[:, t : t + CH], start=True, stop=True)
                h = hpool.tile([H, TB], dt.float32r, tag="h")
                nc.scalar.activation(h[:], z[:], AF.Tanh)
                for i in range(3):
                    z = pp.tile([H, TB], dt.float32, tag="pp")
                    for c, t in chunks():
                        nc.tensor.matmul(z[:, c * CH : (c + 1) * CH],
                                         wms[:, (e * 3 + i) * H : (e * 3 + i + 1) * H],
                                         h[:, c * CH : (c + 1) * CH],
                                         start=True, stop=True)
                    bias = ebm_sb[:, e * 3 + i : e * 3 + i + 1]
                    if i < 2:
                        hn = hpool.tile([H, TB], dt.float32r, tag="h")
                        nc.scalar.activation(hn[:], z[:], AF.Tanh, bias=bias)
                        h = hn
                    else:
                        nc.scalar.activation(h4all[:, e * TB : (e + 1) * TB],
                                             z[:], AF.Tanh, bias=bias)

            # ---------------- combine ----------------
            eo = pp.tile([E, TB], dt.float32, tag="pp")
            for c, t in chunks():
                sl = eo[:, c * CH : (c + 1) * CH]
                # prefill with ebo (broadcast via ones row), then accumulate
                nc.tensor.matmul(sl, ebo_row[:], onesch[:],
                                 start=True, stop=False)
                for e in range(E):
                    nc.tensor.matmul(sl, wos[:, e * E : (e + 1) * E],
                                     h4all[:, e * TB + c * CH : e * TB + (c + 1) * CH],
                                     start=False, stop=(e == E - 1))
            ge = spool.tile([E, TB], dt.float32r, tag="s1")
            nc.vector.tensor_tensor(ge[:], eo[:], gates[:], A.mult)
            yp = pp.tile([1, TB], dt.float32, tag="pp")
            for c, t in chunks():
                nc.tensor.matmul(yp[:, c * CH : (c + 1) * CH], ones16[:],
                                 ge[:, c * CH : (c + 1) * CH],
                                 start=True, stop=True)
            ysb = spool.tile([1, TB], dt.float32, tag="s2")
            nc.vector.tensor_copy(ysb[:], yp[:])
            nc.sync.dma_start(
                y_d[:].rearrange("n o -> o n")[:, t0 : t0 + TB], ysb[:]
            )

        for p in (pp, spool, gpool, h4pool, hpool, wpool):
            p.release()

    _split_excess_waits(nc, mybir)
    return nc


def kernel(**inputs):
    from concourse.bass_utils import run_bass_kernel_spmd

    nc = _CACHE.get("nc")
    if nc is None:
        nc = _CACHE["nc"] = _build_program()

    x = np.ascontiguousarray(np.asarray(inputs["x"], dtype=np.float32))
    shared = {
        k: np.ascontiguousarray(np.asarray(inputs[k], dtype=np.float32))
        for k in ("ew1", "eb1", "ewm", "ebm", "ewo", "ebo",
                  "gw1", "gb1", "gwm", "gbm", "gwo", "gbo")
    }
    in_maps = [
        {"x": x[i * NTOK : (i + 1) * NTOK], **shared} for i in range(NCORES)
    ]
    res = run_bass_kernel_spmd(nc, in_maps, list(range(NCORES)))
    y = np.concatenate([res.results[i]["y"] for i in range(NCORES)], axis=0)
    return y.astype(np.float32)
